# revision 1
# baseline (speedup 1.0000x reference)
"""Trainium2 Bass kernel for a ResNet bottleneck block (training-mode BN).

Computes, for x of shape (64, 1024, 14, 14):
    y1 = relu(bn(conv1x1(x, w1)))        # 1024 -> 256
    y2 = relu(bn(conv3x3(y1, w2)))       # 256 -> 256, pad 1
    z3 = bn(conv1x1(y2, w3))             # 256 -> 1024
    out = relu(x + softplus(residual_scale) * z3)

BN is training-mode: per-channel mean/var over (N, H, W) of the full batch.

Sharding: data-parallel over the batch dim. Each of the 8 NeuronCores gets
8 images. BN batch statistics are made exact by AllGather-ing per-core
per-channel (sum, sum-of-squares) vectors and reducing them on every core.

Conv bias is dropped: training-mode BN of (conv + b) is invariant to b
(it shifts the mean by exactly b). softplus(residual_scale) is folded into
gamma3/beta3 on the host.

All matmuls run in bf16 (fp32 PSUM accumulation). Activations between convs
are kept in bf16; BN statistics and the BN parameter math are fp32.
"""

import os
import numpy as np
import ml_dtypes
from contextlib import ExitStack

import concourse.bass as bass
import concourse.bacc as bacc
import concourse.mybir as mybir
import concourse.tile as tile
from concourse.bass_utils import run_bass_kernel_spmd

F32 = mybir.dt.float32
BF16 = mybir.dt.bfloat16
AX = mybir.AxisListType
ALU = mybir.AluOpType
ACTF = mybir.ActivationFunctionType

N_CORES = 8
N, CIN, H, W = 64, 1024, 14, 14
P = 256
COUT = 1024
NL = N // N_CORES          # images per core (8)
HW = H * W                 # 196
F = NL * HW                # free positions per core (1568)
FT = 4                     # free-dim tiles
FTS = F // FT              # 392 positions per tile (= 2 images)
IPT = NL // FT             # images per free tile (2)
CI_CH = CIN // 128         # 8
P_CH = P // 128            # 2
CO_CH = COUT // 128        # 8
EPS = 1e-5
COUNT = N * HW             # global positions per channel (12544)
PAD = 16                   # padded spatial stride (16x16 per image)
PADSZ = PAD * PAD          # 256


def _emit_stats_exchange(nc, tc, ctx, name, st_local, n_ch):
    """AllGather per-core (sum, sumsq) stats and reduce over cores.

    st_local: SBUF [128, 2, n_ch] fp32 (stat, channel-chunk).
    Returns SBUF [128, 2, n_ch] fp32 of globally-reduced stats.
    """
    cc_mode = os.environ.get("KERNEL_CC_MODE", "ag")
    dram = ctx.enter_context(tc.tile_pool(name=f"{name}_dram", bufs=1, space="DRAM"))
    sb = ctx.enter_context(tc.tile_pool(name=f"{name}_sb", bufs=1))

    cc_in = dram.tile([2, n_ch, 128], F32, name=f"{name}_in")
    cc_out = dram.tile([N_CORES, 2, n_ch, 128], F32,
                       addr_space="Shared" if cc_mode == "ag" else "Local",
                       name=f"{name}_out")
    # SBUF [128, 2, n_ch] -> DRAM [2, n_ch, 128]; reorder DRAM dims to
    # (p, s, c) so the element order matches the SBUF side.
    nc.scalar.dma_start(cc_in.rearrange("s c p -> p s c"), st_local[:])
    if cc_mode == "ag":
        nc.gpsimd.collective_compute(
            "AllGather",
            ALU.bypass,
            replica_groups=[list(range(N_CORES))],
            ins=[cc_in.opt()],
            outs=[cc_out.opt()],
        )
    else:
        # debug: replicate local stats into every row (== BN with local
        # batch stats; sums scale by 8 and /COUNT cancels it exactly)
        for r in range(N_CORES):
            nc.sync.dma_start(cc_out[r], cc_in[:])
    # [r, s, c, p] -> SBUF [p, r, (s c)]: the (s c) pair merges into one
    # DMA dim, keeping the access pattern within the 3-dim DMA limit.
    gath = sb.tile([128, N_CORES, 2 * n_ch], F32, name=f"{name}_gath")
    nc.scalar.dma_start(gath[:], cc_out.rearrange("r s c p -> p r (s c)"))
    red = sb.tile([128, 2, n_ch], F32, name=f"{name}_red")
    nc.vector.tensor_reduce(
        red.rearrange("p s c -> p (s c)"),
        gath.rearrange("p r x -> p x r"),
        axis=AX.X, op=ALU.add,
    )
    return red


def _emit_bn_params(nc, tc, ctx, name, red, gamma, beta, n_ch):
    """From global (sum, sumsq) stats compute per-channel affine (a, b):
    bn(v) = a * z + b  where z is the conv output without bias.
    a = gamma / sqrt(var + eps), b = beta - a * mean.
    Returns (a, b): SBUF [128, n_ch] fp32 each.
    """
    sb = ctx.enter_context(tc.tile_pool(name=f"{name}_bn", bufs=1))
    inv_n = 1.0 / COUNT

    # Short serial chain (each link costs ~2us of sem/dispatch latency on the
    # stage boundary): fold the 1/n scalings into fused ops.
    epst = sb.tile([128, 1], F32, name=f"{name}_eps")
    nc.vector.memset(epst[:], EPS)  # dep-free, runs early
    sums = red[:, 0, :]
    ssq = red[:, 1, :]
    s2 = sb.tile([128, n_ch], F32, name=f"{name}_s2")
    nc.vector.tensor_mul(s2[:], sums, sums)
    # nvn = sum^2/n - ssq  (= -n*var)
    nvn = sb.tile([128, n_ch], F32, name=f"{name}_nvn")
    nc.vector.scalar_tensor_tensor(
        nvn[:], s2[:], inv_n, ssq, op0=ALU.mult, op1=ALU.subtract
    )
    # std = sqrt(nvn * (-1/n) + eps) = sqrt(var + eps)
    std = sb.tile([128, n_ch], F32, name=f"{name}_std")
    nc.scalar.activation(std[:], nvn[:], ACTF.Sqrt, bias=epst[:, 0:1],
                         scale=-inv_n)
    rstd = sb.tile([128, n_ch], F32, name=f"{name}_rstd")
    nc.vector.reciprocal(rstd[:], std[:])
    a = sb.tile([128, n_ch], F32, name=f"{name}_a")
    nc.vector.tensor_mul(a[:], gamma[:], rstd[:])
    asum = sb.tile([128, n_ch], F32, name=f"{name}_asum")
    nc.vector.tensor_mul(asum[:], a[:], sums)
    # b = beta - a*sum/n
    b = sb.tile([128, n_ch], F32, name=f"{name}_b")
    nc.vector.scalar_tensor_tensor(
        b[:], asum[:], -inv_n, beta[:], op0=ALU.mult, op1=ALU.add
    )
    return a, b


def build_debug(stage):
    """Structural-bisect builds: stage '0' = DMA passthrough only,
    '1' = +conv1/BN1, '2' = +conv2/BN2."""
    nc = bacc.Bacc("TRN2", target_bir_lowering=False, debug=False,
                   num_devices=N_CORES)
    x_d = nc.dram_tensor("x", [NL, CIN, HW], F32, kind="ExternalInput")
    w1_d = nc.dram_tensor("w1t", [CI_CH, 128, P], BF16, kind="ExternalInput")
    w2_d = nc.dram_tensor("w2t", [P_CH, 128, 9, P], BF16, kind="ExternalInput")
    w3_d = nc.dram_tensor("w3t", [P_CH, 128, COUT], BF16, kind="ExternalInput")
    gb1_d = nc.dram_tensor("gb1", [2, P_CH, 128], F32, kind="ExternalInput")
    gb2_d = nc.dram_tensor("gb2", [2, P_CH, 128], F32, kind="ExternalInput")
    gb3_d = nc.dram_tensor("gb3", [2, CO_CH, 128], F32, kind="ExternalInput")
    out_d = nc.dram_tensor("out", [NL, CIN, HW], F32, kind="ExternalOutput")

    with tile.TileContext(nc) as tc, ExitStack() as ctx:
        xpool = ctx.enter_context(tc.tile_pool(name="xpool", bufs=1))
        consts = ctx.enter_context(tc.tile_pool(name="consts", bufs=1))
        statp = ctx.enter_context(tc.tile_pool(name="statp", bufs=1))
        scrp = ctx.enter_context(tc.tile_pool(name="scrp", bufs=2))
        psum = ctx.enter_context(tc.tile_pool(name="psum", bufs=8, space="PSUM"))

        xf = [xpool.tile([128, F], F32, name=f"xf{c}") for c in range(CI_CH)]
        for c in range(CI_CH):
            nc.sync.dma_start(
                xf[c][:], x_d[:, c * 128:(c + 1) * 128, :].rearrange("n p f -> p n f")
            )
        if stage == "0":
            for c in range(CI_CH):
                nc.sync.dma_start(
                    out_d[:, c * 128:(c + 1) * 128, :].rearrange("n p f -> p n f"),
                    xf[c][:],
                )
        elif stage == "1d":
            # no matmuls: bf16 convert then ACT copy back to f32, DMA out
            xb = [xpool.tile([128, F], BF16, name=f"xb{c}") for c in range(CI_CH)]
            zz = [xpool.tile([128, F], F32, name=f"zz{c}") for c in range(CI_CH)]
            for c in range(CI_CH):
                if c % 2 == 0:
                    nc.vector.tensor_copy(xb[c][:], xf[c][:])
                else:
                    nc.scalar.copy(xb[c][:], xf[c][:])
                nc.scalar.copy(zz[c][:], xb[c][:])
                nc.sync.dma_start(
                    out_d[:, c * 128:(c + 1) * 128, :].rearrange("n p f -> p n f"),
                    zz[c][:],
                )
        elif stage == "1c":
            # conv1 with NON-interleaved accumulation groups
            w1sb = [consts.tile([128, P], BF16, name=f"w1sb{c}")
                    for c in range(CI_CH)]
            for c in range(CI_CH):
                nc.sync.dma_start(w1sb[c][:], w1_d[c])
            xb = [xpool.tile([128, F], BF16, name=f"xb{c}") for c in range(CI_CH)]
            for c in range(CI_CH):
                if c % 2 == 0:
                    nc.vector.tensor_copy(xb[c][:], xf[c][:])
                else:
                    nc.scalar.copy(xb[c][:], xf[c][:])
            z1 = [xpool.tile([128, F], F32, name=f"z1_{c}") for c in range(P_CH)]
            for co in range(P_CH):
                for ft in range(FT):
                    pt = psum.tile([128, FTS], F32, name="pt", tag="pt")
                    for ci in range(CI_CH):
                        nc.tensor.matmul(
                            pt[:],
                            w1sb[ci][:, co * 128:(co + 1) * 128],
                            xb[ci][:, ft * FTS:(ft + 1) * FTS],
                            start=(ci == 0), stop=(ci == CI_CH - 1),
                        )
                    nc.scalar.activation(
                        z1[co][:, ft * FTS:(ft + 1) * FTS], pt[:], ACTF.Copy)
            for c in range(P_CH):
                nc.sync.dma_start(
                    out_d[:, c * 128:(c + 1) * 128, :].rearrange("n p f -> p n f"),
                    z1[c][:],
                )
        elif stage in ("1a", "1b", "1e", "1f"):
            # sequential-group conv1 plus one feature at a time:
            # 1a: plain ACT copy out (== 1c)
            # 1b: + ACT accum_out sums
            # 1e: + DVE TTR sumsq
            # 1f: + stats reduce (no collective, no BN)
            w1sb = [consts.tile([128, P], BF16, name=f"w1sb{c}")
                    for c in range(CI_CH)]
            for c in range(CI_CH):
                nc.sync.dma_start(w1sb[c][:], w1_d[c])
            xb = [xpool.tile([128, F], BF16, name=f"xb{c}") for c in range(CI_CH)]
            for c in range(CI_CH):
                if c % 2 == 0:
                    nc.vector.tensor_copy(xb[c][:], xf[c][:])
                else:
                    nc.scalar.copy(xb[c][:], xf[c][:])
            z1 = [xpool.tile([128, F], F32, name=f"z1_{c}") for c in range(P_CH)]
            s1p = statp.tile([128, P_CH, FT], F32, name="s1p")
            q1p = statp.tile([128, P_CH, FT], F32, name="q1p")
            for co in range(P_CH):
                for ft in range(FT):
                    pt = psum.tile([128, FTS], F32, name="pt", tag="pt")
                    for ci in range(CI_CH):
                        nc.tensor.matmul(
                            pt[:],
                            w1sb[ci][:, co * 128:(co + 1) * 128],
                            xb[ci][:, ft * FTS:(ft + 1) * FTS],
                            start=(ci == 0), stop=(ci == CI_CH - 1),
                        )
                    zs = z1[co][:, ft * FTS:(ft + 1) * FTS]
                    if stage == "1a":
                        nc.scalar.activation(zs, pt[:], ACTF.Copy)
                    else:
                        nc.scalar.activation(
                            zs, pt[:], ACTF.Copy,
                            accum_out=s1p[:, co, ft:ft + 1])
                    if stage in ("1e", "1f"):
                        sq = scrp.tile([128, FTS], F32, name="sq", tag="sq")
                        nc.vector.tensor_mul(sq[:], zs, zs)
                        nc.vector.tensor_reduce(
                            q1p[:, co, ft:ft + 1], sq[:], axis=AX.X, op=ALU.add)
            if stage == "1f":
                st1 = statp.tile([128, 2, P_CH], F32, name="st1")
                nc.vector.tensor_reduce(st1[:, 0, :], s1p[:], axis=AX.X,
                                        op=ALU.add)
                nc.vector.tensor_reduce(st1[:, 1, :], q1p[:], axis=AX.X,
                                        op=ALU.add)
                stout = xpool.tile([128, 4], F32, name="stout")
                nc.vector.tensor_copy(stout[:], st1.rearrange("p s c -> p (s c)"))
            for c in range(P_CH):
                nc.sync.dma_start(
                    out_d[:, c * 128:(c + 1) * 128, :].rearrange("n p f -> p n f"),
                    z1[c][:],
                )
        elif stage == "1":
            # conv1 + stats + BN1 apply, result written to out chunks 0..1
            w1sb = [consts.tile([128, P], BF16, name=f"w1sb{c}")
                    for c in range(CI_CH)]
            for c in range(CI_CH):
                nc.sync.dma_start(w1sb[c][:], w1_d[c])
            g1 = consts.tile([128, P_CH], F32, name="g1")
            be1 = consts.tile([128, P_CH], F32, name="be1")
            nc.sync.dma_start(g1[:], gb1_d[0].rearrange("c p -> p c"))
            nc.sync.dma_start(be1[:], gb1_d[1].rearrange("c p -> p c"))
            xb = [xpool.tile([128, F], BF16, name=f"xb{c}") for c in range(CI_CH)]
            for c in range(CI_CH):
                if c % 2 == 0:
                    nc.vector.tensor_copy(xb[c][:], xf[c][:])
                else:
                    nc.scalar.copy(xb[c][:], xf[c][:])
            z1 = [xpool.tile([128, F], F32, name=f"z1_{c}") for c in range(P_CH)]
            s1p = statp.tile([128, P_CH, FT], F32, name="s1p")
            q1p = statp.tile([128, P_CH, FT], F32, name="q1p")
            for co in range(P_CH):
                for ft in range(FT):
                    pt = psum.tile([128, FTS], F32, name="pt", tag="pt")
                    for ci in range(CI_CH):
                        nc.tensor.matmul(
                            pt[:],
                            w1sb[ci][:, co * 128:(co + 1) * 128],
                            xb[ci][:, ft * FTS:(ft + 1) * FTS],
                            start=(ci == 0), stop=(ci == CI_CH - 1),
                        )
                    zs = z1[co][:, ft * FTS:(ft + 1) * FTS]
                    nc.scalar.activation(
                        zs, pt[:], ACTF.Copy, accum_out=s1p[:, co, ft:ft + 1])
                    sq = scrp.tile([128, FTS], F32, name="sq", tag="sq")
                    nc.vector.tensor_mul(sq[:], zs, zs)
                    nc.vector.tensor_reduce(
                        q1p[:, co, ft:ft + 1], sq[:], axis=AX.X, op=ALU.add)
            st1 = statp.tile([128, 2, P_CH], F32, name="st1")
            nc.vector.tensor_reduce(st1[:, 0, :], s1p[:], axis=AX.X, op=ALU.add)
            nc.vector.tensor_reduce(st1[:, 1, :], q1p[:], axis=AX.X, op=ALU.add)
            red1 = _emit_stats_exchange(nc, tc, ctx, "bn1", st1, P_CH)
            a1, b1 = _emit_bn_params(nc, tc, ctx, "bn1", red1, g1, be1, P_CH)
            y1 = [xpool.tile([128, F], F32, name=f"y1_{c}") for c in range(P_CH)]
            for c in range(P_CH):
                nc.scalar.activation(y1[c][:], z1[c][:], ACTF.Relu,
                                     bias=b1[:, c:c + 1], scale=a1[:, c:c + 1])
            for c in range(P_CH):
                nc.sync.dma_start(
                    out_d[:, c * 128:(c + 1) * 128, :].rearrange("n p f -> p n f"),
                    y1[c][:],
                )
        elif stage == "2":
            # conv1 -> BN1 -> padded y1p -> conv2 -> BN2(local dma exchange)
            w1sb = [consts.tile([128, P], BF16, name=f"w1sb{c}")
                    for c in range(CI_CH)]
            for c in range(CI_CH):
                nc.sync.dma_start(w1sb[c][:], w1_d[c])
            w2sb = [consts.tile([128, 9, P], BF16, name=f"w2sb{c}")
                    for c in range(P_CH)]
            for c in range(P_CH):
                nc.sync.dma_start(w2sb[c][:], w2_d[c])
            g1 = consts.tile([128, P_CH], F32, name="g1")
            be1 = consts.tile([128, P_CH], F32, name="be1")
            nc.sync.dma_start(g1[:], gb1_d[0].rearrange("c p -> p c"))
            nc.sync.dma_start(be1[:], gb1_d[1].rearrange("c p -> p c"))
            g2 = consts.tile([128, P_CH], F32, name="g2")
            be2 = consts.tile([128, P_CH], F32, name="be2")
            nc.sync.dma_start(g2[:], gb2_d[0].rearrange("c p -> p c"))
            nc.sync.dma_start(be2[:], gb2_d[1].rearrange("c p -> p c"))
            xb = [xpool.tile([128, F], BF16, name=f"xb{c}") for c in range(CI_CH)]
            for c in range(CI_CH):
                if c % 2 == 0:
                    nc.vector.tensor_copy(xb[c][:], xf[c][:])
                else:
                    nc.scalar.copy(xb[c][:], xf[c][:])
            y1p = [xpool.tile([128, NL, PAD, PAD], BF16, name=f"y1p{c}")
                   for c in range(P_CH)]
            for c in range(P_CH):
                nc.gpsimd.memset(y1p[c][:], 0)
            z1 = [xpool.tile([128, F], F32, name=f"z1_{c}") for c in range(P_CH)]
            z2 = [xpool.tile([128, F], F32, name=f"z2_{c}") for c in range(P_CH)]
            s1p = statp.tile([128, P_CH, FT], F32, name="s1p")
            q1p = statp.tile([128, P_CH, FT], F32, name="q1p")
            for co in range(P_CH):
                for ft in range(FT):
                    pt = psum.tile([128, FTS], F32, name="pt", tag="pt")
                    for ci in range(CI_CH):
                        nc.tensor.matmul(
                            pt[:],
                            w1sb[ci][:, co * 128:(co + 1) * 128],
                            xb[ci][:, ft * FTS:(ft + 1) * FTS],
                            start=(ci == 0), stop=(ci == CI_CH - 1),
                        )
                    zs = z1[co][:, ft * FTS:(ft + 1) * FTS]
                    nc.scalar.activation(
                        zs, pt[:], ACTF.Copy, accum_out=s1p[:, co, ft:ft + 1])
                    sq = scrp.tile([128, FTS], F32, name="sq", tag="sq")
                    nc.vector.tensor_mul(sq[:], zs, zs)
                    nc.vector.tensor_reduce(
                        q1p[:, co, ft:ft + 1], sq[:], axis=AX.X, op=ALU.add)
            st1 = statp.tile([128, 2, P_CH], F32, name="st1")
            nc.vector.tensor_reduce(st1[:, 0, :], s1p[:], axis=AX.X, op=ALU.add)
            nc.vector.tensor_reduce(st1[:, 1, :], q1p[:], axis=AX.X, op=ALU.add)
            red1 = _emit_stats_exchange(nc, tc, ctx, "bn1", st1, P_CH)
            a1, b1 = _emit_bn_params(nc, tc, ctx, "bn1", red1, g1, be1, P_CH)
            for c in range(P_CH):
                nc.scalar.activation(
                    y1p[c][:, :, 1:1 + H, 1:1 + W],
                    z1[c].rearrange("p (n h w) -> p n h w", n=NL, h=H, w=W),
                    ACTF.Relu, bias=b1[:, c:c + 1], scale=a1[:, c:c + 1])
            s2p = statp.tile([128, P_CH, FT], F32, name="s2p")
            q2p = statp.tile([128, P_CH, FT], F32, name="q2p")
            for co in range(P_CH):
                for ft in range(FT):
                    pt = psum.tile([128, FTS], F32, name="pt", tag="pt")
                    for ci in range(P_CH):
                        for tap in range(9):
                            ky, kx = divmod(tap, 3)
                            nc.tensor.matmul(
                                pt[:],
                                w2sb[ci][:, tap, co * 128:(co + 1) * 128],
                                y1p[ci][:, ft * IPT:(ft + 1) * IPT,
                                        ky:ky + H, kx:kx + W],
                                start=(ci == 0 and tap == 0),
                                stop=(ci == P_CH - 1 and tap == 8),
                            )
                    zs = z2[co][:, ft * FTS:(ft + 1) * FTS]
                    nc.scalar.activation(
                        zs, pt[:], ACTF.Copy, accum_out=s2p[:, co, ft:ft + 1])
                    sq = scrp.tile([128, FTS], F32, name="sq", tag="sq")
                    nc.vector.tensor_mul(sq[:], zs, zs)
                    nc.vector.tensor_reduce(
                        q2p[:, co, ft:ft + 1], sq[:], axis=AX.X, op=ALU.add)
            st2 = statp.tile([128, 2, P_CH], F32, name="st2")
            nc.vector.tensor_reduce(st2[:, 0, :], s2p[:], axis=AX.X, op=ALU.add)
            nc.vector.tensor_reduce(st2[:, 1, :], q2p[:], axis=AX.X, op=ALU.add)
            red2 = _emit_stats_exchange(nc, tc, ctx, "bn2", st2, P_CH)
            a2, b2 = _emit_bn_params(nc, tc, ctx, "bn2", red2, g2, be2, P_CH)
            y2 = [xpool.tile([128, F], F32, name=f"y2_{c}") for c in range(P_CH)]
            for c in range(P_CH):
                nc.scalar.activation(y2[c][:], z2[c][:], ACTF.Relu,
                                     bias=b2[:, c:c + 1], scale=a2[:, c:c + 1])
            for c in range(P_CH):
                nc.sync.dma_start(
                    out_d[:, c * 128:(c + 1) * 128, :].rearrange("n p f -> p n f"),
                    y2[c][:],
                )
    nc.compile()
    return nc


def build():
    nc = bacc.Bacc("TRN2", target_bir_lowering=False, debug=False,
                   num_devices=N_CORES)

    # ---- I/O -------------------------------------------------------------
    x_d = nc.dram_tensor("x", [NL, CIN, HW], F32, kind="ExternalInput")
    xb_d = nc.dram_tensor("xb16", [CI_CH, 128, F], BF16, kind="ExternalInput")
    w1_d = nc.dram_tensor("w1t", [CI_CH, 128, P], BF16, kind="ExternalInput")
    w2_d = nc.dram_tensor("w2t", [P_CH, 128, 9, P], BF16, kind="ExternalInput")
    w3_d = nc.dram_tensor("w3t", [P_CH, 128, COUT], BF16, kind="ExternalInput")
    gb1_d = nc.dram_tensor("gb1", [2, P_CH, 128], F32, kind="ExternalInput")
    gb2_d = nc.dram_tensor("gb2", [2, P_CH, 128], F32, kind="ExternalInput")
    gb3_d = nc.dram_tensor("gb3", [2, CO_CH, 128], F32, kind="ExternalInput")
    out_d = nc.dram_tensor("out", [NL, CIN, HW], F32, kind="ExternalOutput")

    with tile.TileContext(nc) as tc, ExitStack() as ctx:
        consts = ctx.enter_context(tc.tile_pool(name="consts", bufs=1))
        xpool = ctx.enter_context(tc.tile_pool(name="xpool", bufs=1))
        actp = ctx.enter_context(tc.tile_pool(name="actp", bufs=1))
        statp = ctx.enter_context(tc.tile_pool(name="statp", bufs=1))
        scrp = ctx.enter_context(tc.tile_pool(name="scrp", bufs=2))
        psum = ctx.enter_context(tc.tile_pool(name="psum", bufs=8, space="PSUM"))

        # ---- bf16 x first: it gates conv1, so its DMAs lead the queue ----
        xb = [xpool.tile([128, F], BF16, name=f"xz{c}", tag=f"xz{c}")
              for c in range(CI_CH)]
        for c in range(CI_CH):
            nc.sync.dma_start(xb[c][:], xb_d[c])

        # ---- weights + BN consts ----------------------------------------
        w1sb = [consts.tile([128, P], BF16, name=f"w1sb{c}") for c in range(CI_CH)]
        for c in range(CI_CH):
            nc.sync.dma_start(w1sb[c][:], w1_d[c])
        w2sb = [consts.tile([128, 9, P], BF16, name=f"w2sb{c}") for c in range(P_CH)]
        for c in range(P_CH):
            nc.sync.dma_start(w2sb[c][:], w2_d[c])
        w3sb = [consts.tile([128, COUT], BF16, name=f"w3sb{c}") for c in range(P_CH)]
        for c in range(P_CH):
            nc.sync.dma_start(w3sb[c][:], w3_d[c])

        g1 = consts.tile([128, P_CH], F32, name="g1")
        be1 = consts.tile([128, P_CH], F32, name="be1")
        g2 = consts.tile([128, P_CH], F32, name="g2")
        be2 = consts.tile([128, P_CH], F32, name="be2")
        g3 = consts.tile([128, CO_CH], F32, name="g3")
        be3 = consts.tile([128, CO_CH], F32, name="be3")
        for t, d in ((g1, gb1_d), (g2, gb2_d), (g3, gb3_d)):
            nc.sync.dma_start(t[:], d[0].rearrange("c p -> p c"))
        for t, d in ((be1, gb1_d), (be2, gb2_d), (be3, gb3_d)):
            nc.sync.dma_start(t[:], d[1].rearrange("c p -> p c"))

        # fp32 x is only needed by the residual tail; emitted after xb and
        # the weights so its bulk DMAs stay off the conv1 critical path.
        xf = [xpool.tile([128, F], F32, name=f"xf{c}") for c in range(CI_CH)]
        for c in range(CI_CH):
            nc.sync.dma_start(
                xf[c][:], x_d[:, c * 128:(c + 1) * 128, :].rearrange("n p f -> p n f")
            )

        # padded bf16 activations for the 3x3 conv: [128, NL, 16, 16]
        y1p = [actp.tile([128, NL, PAD, PAD], BF16, name=f"y1p{c}") for c in range(P_CH)]
        for c in range(P_CH):
            nc.gpsimd.memset(y1p[c][:], 0)

        z1 = [actp.tile([128, F], F32, name=f"z1_{c}") for c in range(P_CH)]
        z2 = [actp.tile([128, F], F32, name=f"z2_{c}") for c in range(P_CH)]
        y2 = [actp.tile([128, F], BF16, name=f"y2_{c}") for c in range(P_CH)]

        # ================= stage A: conv1 (1x1, 1024 -> 256) =============
        # NOTE: accumulation groups are kept sequential per PSUM tile
        # (group-outer, contraction-inner). Interleaving groups across
        # banks (ci-outer emission) hangs on hardware.
        s1p = statp.tile([128, P_CH, FT], F32, name="s1p")
        st1 = statp.tile([128, 2, P_CH], F32, name="st1")
        for co in range(P_CH):
            for ft in range(FT):
                pt = psum.tile([128, FTS], F32, name="pt", tag="pt")
                for ci in range(CI_CH):
                    nc.tensor.matmul(
                        pt[:],
                        w1sb[ci][:, co * 128:(co + 1) * 128],
                        xb[ci][:, ft * FTS:(ft + 1) * FTS],
                        start=(ci == 0),
                        stop=(ci == CI_CH - 1),
                    )
                # PSUM -> SBUF copy with per-channel running sum (DVE), and
                # square with running sum-of-squares (ACT). Each instruction
                # reads PSUM through exactly one non-scalar input (walrus
                # rejects two PSUM reads on one DVE op).
                zs = z1[co][:, ft * FTS:(ft + 1) * FTS]
                nc.scalar.activation(
                    zs, pt[:], ACTF.Copy, accum_out=s1p[:, co, ft:ft + 1],
                )
            sq = scrp.tile([128, F], BF16, name="sq", tag="sq")
            nc.vector.tensor_mul(sq[:], z1[co][:], z1[co][:])
            nc.vector.tensor_reduce(
                st1[:, 1, co:co + 1], sq[:], axis=AX.X, op=ALU.add,
            )
        nc.vector.tensor_reduce(st1[:, 0, :], s1p[:], axis=AX.X, op=ALU.add)

        red1 = _emit_stats_exchange(nc, tc, ctx, "bn1", st1, P_CH)
        a1, b1 = _emit_bn_params(nc, tc, ctx, "bn1", red1, g1, be1, P_CH)

        for c in range(P_CH):
            nc.scalar.activation(
                y1p[c][:, :, 1:1 + H, 1:1 + W],
                z1[c].rearrange("p (n h w) -> p n h w", n=NL, h=H, w=W),
                ACTF.Relu,
                bias=b1[:, c:c + 1],
                scale=a1[:, c:c + 1],
            )

        # ================= stage B: conv2 (3x3, 256 -> 256) ==============
        s2p = statp.tile([128, P_CH, FT], F32, name="s2p")
        st2 = statp.tile([128, 2, P_CH], F32, name="st2")
        for co in range(P_CH):
            for ft in range(FT):
                pt = psum.tile([128, FTS], F32, name="pt", tag="pt")
                for ci in range(P_CH):
                    for tap in range(9):
                        ky, kx = divmod(tap, 3)
                        nc.tensor.matmul(
                            pt[:],
                            w2sb[ci][:, tap, co * 128:(co + 1) * 128],
                            y1p[ci][:, ft * IPT:(ft + 1) * IPT,
                                    ky:ky + H, kx:kx + W],
                            start=(ci == 0 and tap == 0),
                            stop=(ci == P_CH - 1 and tap == 8),
                        )
                zs = z2[co][:, ft * FTS:(ft + 1) * FTS]
                nc.scalar.activation(
                    zs, pt[:], ACTF.Copy, accum_out=s2p[:, co, ft:ft + 1],
                )
            sq = scrp.tile([128, F], BF16, name="sq", tag="sq")
            nc.vector.tensor_mul(sq[:], z2[co][:], z2[co][:])
            nc.vector.tensor_reduce(
                st2[:, 1, co:co + 1], sq[:], axis=AX.X, op=ALU.add,
            )
        nc.vector.tensor_reduce(st2[:, 0, :], s2p[:], axis=AX.X, op=ALU.add)

        red2 = _emit_stats_exchange(nc, tc, ctx, "bn2", st2, P_CH)
        a2, b2 = _emit_bn_params(nc, tc, ctx, "bn2", red2, g2, be2, P_CH)

        for c in range(P_CH):
            nc.scalar.activation(
                y2[c][:], z2[c][:], ACTF.Relu,
                bias=b2[:, c:c + 1], scale=a2[:, c:c + 1],
            )

        # ================= stage C: conv3 (1x1, 256 -> 1024) =============
        s3p = statp.tile([128, CO_CH, FT], F32, name="s3p")
        st3 = statp.tile([128, 2, CO_CH], F32, name="st3")
        z3 = [xpool.tile([128, F], F32, name=f"xz{c}", tag=f"xz{c}")
              for c in range(CO_CH)]
        for co in range(CO_CH):
            for ft in range(FT):
                pt = psum.tile([128, FTS], F32, name="pt", tag="pt")
                for ci in range(P_CH):
                    nc.tensor.matmul(
                        pt[:],
                        w3sb[ci][:, co * 128:(co + 1) * 128],
                        y2[ci][:, ft * FTS:(ft + 1) * FTS],
                        start=(ci == 0),
                        stop=(ci == P_CH - 1),
                    )
                zs = z3[co][:, ft * FTS:(ft + 1) * FTS]
                nc.scalar.activation(
                    zs, pt[:], ACTF.Copy, accum_out=s3p[:, co, ft:ft + 1],
                )
            sq = scrp.tile([128, F], BF16, name="sq", tag="sq")
            nc.vector.tensor_mul(sq[:], z3[co][:], z3[co][:])
            nc.vector.tensor_reduce(
                st3[:, 1, co:co + 1], sq[:], axis=AX.X, op=ALU.add,
            )
        nc.vector.tensor_reduce(st3[:, 0, :], s3p[:], axis=AX.X, op=ALU.add)

        red3 = _emit_stats_exchange(nc, tc, ctx, "bn3", st3, CO_CH)
        a3, b3 = _emit_bn_params(nc, tc, ctx, "bn3", red3, g3, be3, CO_CH)

        # tail: out = relu((a3*z3 + x) + b3) in two fused passes per chunk.
        # Result lands in xf (fp32, dead after the STT reads it) so the
        # output DMA stays fp32 while z3 itself is bf16.
        for co in range(CO_CH):
            nc.vector.scalar_tensor_tensor(
                xf[co][:], z3[co][:], a3[:, co:co + 1], xf[co][:],
                op0=ALU.mult, op1=ALU.add,
            )
            nc.scalar.activation(
                xf[co][:], xf[co][:], ACTF.Relu, bias=b3[:, co:co + 1],
            )
            deng = nc.sync if co % 2 == 0 else nc.scalar
            deng.dma_start(
                out_d[:, co * 128:(co + 1) * 128, :].rearrange("n p f -> p n f"),
                xf[co][:],
            )
    nc.compile()
    return nc


_NC_CACHE = None


def _get_nc():
    global _NC_CACHE
    if _NC_CACHE is None:
        stage = os.environ.get("KERNEL_STAGE", "full")
        _NC_CACHE = build() if stage == "full" else build_debug(stage)
    return _NC_CACHE


def _prep_host(w1, w2, w3, g1, be1, g2, be2, g3, be3, residual_scale):
    bf = ml_dtypes.bfloat16
    # conv weights, pre-transposed to [ci, ...] layouts for lhsT
    w1t = np.ascontiguousarray(
        w1.reshape(P, CIN).T.astype(bf)).reshape(CI_CH, 128, P)
    w2t = np.ascontiguousarray(
        w2.transpose(1, 2, 3, 0).astype(bf)).reshape(P_CH, 128, 9, P)
    w3t = np.ascontiguousarray(
        w3.reshape(COUT, P).T.astype(bf)).reshape(P_CH, 128, COUT)
    s = np.float32(np.log1p(np.exp(np.float64(residual_scale[0]))))
    gb1 = np.stack([g1, be1]).astype(np.float32).reshape(2, P_CH, 128)
    gb2 = np.stack([g2, be2]).astype(np.float32).reshape(2, P_CH, 128)
    gb3 = (np.stack([g3, be3]) * s).astype(np.float32).reshape(2, CO_CH, 128)
    return w1t, w2t, w3t, gb1, gb2, gb3


def prepare_in_maps(inputs):
    x = np.asarray(inputs["x"], dtype=np.float32)
    w1t, w2t, w3t, gb1, gb2, gb3 = _prep_host(
        np.asarray(inputs["w1"], np.float32), np.asarray(inputs["w2"], np.float32),
        np.asarray(inputs["w3"], np.float32), np.asarray(inputs["g1"], np.float32),
        np.asarray(inputs["be1"], np.float32), np.asarray(inputs["g2"], np.float32),
        np.asarray(inputs["be2"], np.float32), np.asarray(inputs["g3"], np.float32),
        np.asarray(inputs["be3"], np.float32),
        np.asarray(inputs["residual_scale"], np.float32),
    )
    in_maps = []
    for c in range(N_CORES):
        shard = np.ascontiguousarray(
            x[c * NL:(c + 1) * NL].reshape(NL, CIN, HW))
        xb16 = np.ascontiguousarray(
            shard.transpose(1, 0, 2).astype(ml_dtypes.bfloat16)
        ).reshape(CI_CH, 128, F)
        in_maps.append({
            "x": shard, "xb16": xb16, "w1t": w1t, "w2t": w2t, "w3t": w3t,
            "gb1": gb1, "gb2": gb2, "gb3": gb3,
        })
    return in_maps


def kernel(**inputs):
    in_maps = prepare_in_maps(inputs)
    nc = _get_nc()
    trace = bool(int(os.environ.get("KERNEL_PROFILE", "0")))
    try:
        res = run_bass_kernel_spmd(nc, in_maps, list(range(N_CORES)), trace=trace)
    except ModuleNotFoundError:
        # axon NTFF profile hook unavailable in this container
        res = run_bass_kernel_spmd(nc, in_maps, list(range(N_CORES)), trace=False)
    if trace:
        kernel.last_exec_time_ns = getattr(res, "exec_time_ns", None)
        kernel.last_profile = res
    out = np.concatenate([res.results[c]["out"] for c in range(N_CORES)], axis=0)
    return out.reshape(N, CIN, H, W)



# revision 15
# speedup vs baseline: 1.2465x; 1.2465x over previous
"""Trainium2 Bass kernel for a ResNet bottleneck block (training-mode BN).

Computes, for x of shape (64, 1024, 14, 14):
    y1 = relu(bn(conv1x1(x, w1)))        # 1024 -> 256
    y2 = relu(bn(conv3x3(y1, w2)))       # 256 -> 256, pad 1
    z3 = bn(conv1x1(y2, w3))             # 256 -> 1024
    out = relu(x + softplus(residual_scale) * z3)

BN is training-mode: per-channel mean/var over (N, H, W) of the full batch.

Sharding: data-parallel over the batch dim. Each of the 8 NeuronCores gets
8 images. BN batch statistics are made exact by AllGather-ing per-core
per-channel (sum, sum-of-squares) vectors and reducing them on every core.

Conv bias is dropped: training-mode BN of (conv + b) is invariant to b
(it shifts the mean by exactly b). softplus(residual_scale) is folded into
gamma3/beta3 on the host.

All matmuls run in bf16 (fp32 PSUM accumulation). Activations between convs
are kept in bf16; BN statistics and the BN parameter math are fp32.
"""

import os
import numpy as np
import ml_dtypes
from contextlib import ExitStack

import concourse.bass as bass
import concourse.bacc as bacc
import concourse.mybir as mybir
import concourse.tile as tile
from concourse.bass_utils import run_bass_kernel_spmd

F32 = mybir.dt.float32
BF16 = mybir.dt.bfloat16
AX = mybir.AxisListType
ALU = mybir.AluOpType
ACTF = mybir.ActivationFunctionType

N_CORES = 8
N, CIN, H, W = 64, 1024, 14, 14
P = 256
COUT = 1024
NL = N // N_CORES          # images per core (8)
HW = H * W                 # 196
F = NL * HW                # free positions per core (1568)
FT = 4                     # free-dim tiles
FTS = F // FT              # 392 positions per tile (= 2 images)
IPT = NL // FT             # images per free tile (2)
CI_CH = CIN // 128         # 8
P_CH = P // 128            # 2
CO_CH = COUT // 128        # 8
EPS = 1e-5
COUNT = N * HW             # global positions per channel (12544)
PAD = 16                   # padded spatial stride (16x16 per image)
PADSZ = PAD * PAD          # 256


def _emit_stats_exchange(nc, tc, ctx, name, st_local, n_ch):
    """AllGather per-core (sum, sumsq) stats and reduce over cores.

    st_local: SBUF [128, 2, n_ch] fp32 (stat, channel-chunk).
    Returns SBUF [128, 2, n_ch] fp32 of globally-reduced stats.
    """
    cc_mode = os.environ.get("KERNEL_CC_MODE", "ag")
    dram = ctx.enter_context(tc.tile_pool(name=f"{name}_dram", bufs=1, space="DRAM"))
    sb = ctx.enter_context(tc.tile_pool(name=f"{name}_sb", bufs=1))

    cc_in = dram.tile([2, n_ch, 128], F32, name=f"{name}_in")
    cc_out = dram.tile([N_CORES, 2, n_ch, 128], F32,
                       addr_space="Shared" if cc_mode == "ag" else "Local",
                       name=f"{name}_out")
    # SBUF [128, 2, n_ch] -> DRAM [2, n_ch, 128]; reorder DRAM dims to
    # (p, s, c) so the element order matches the SBUF side.
    nc.scalar.dma_start(cc_in.rearrange("s c p -> p s c"), st_local[:])
    if cc_mode == "ag":
        nc.gpsimd.collective_compute(
            "AllGather",
            ALU.bypass,
            replica_groups=[list(range(N_CORES))],
            ins=[cc_in.opt()],
            outs=[cc_out.opt()],
        )
    else:
        # debug: replicate local stats into every row (== BN with local
        # batch stats; sums scale by 8 and /COUNT cancels it exactly)
        for r in range(N_CORES):
            nc.sync.dma_start(cc_out[r], cc_in[:])
    # [r, s, c, p] -> SBUF [p, r, (s c)]: the (s c) pair merges into one
    # DMA dim, keeping the access pattern within the 3-dim DMA limit.
    gath = sb.tile([128, N_CORES, 2 * n_ch], F32, name=f"{name}_gath")
    nc.scalar.dma_start(gath[:], cc_out.rearrange("r s c p -> p r (s c)"))
    red = sb.tile([128, 2, n_ch], F32, name=f"{name}_red")
    nc.vector.tensor_reduce(
        red.rearrange("p s c -> p (s c)"),
        gath.rearrange("p r x -> p x r"),
        axis=AX.X, op=ALU.add,
    )
    return red


def _emit_bn_params(nc, tc, ctx, name, red, gamma, beta, n_ch):
    """From global (sum, sumsq) stats compute per-channel affine (a, b):
    bn(v) = a * z + b  where z is the conv output without bias.
    a = gamma / sqrt(var + eps), b = beta - a * mean.
    Returns (a, b): SBUF [128, n_ch] fp32 each.
    """
    sb = ctx.enter_context(tc.tile_pool(name=f"{name}_bn", bufs=1))
    inv_n = 1.0 / COUNT

    # Short serial chain (each link costs ~2us of sem/dispatch latency on the
    # stage boundary): fold the 1/n scalings into fused ops.
    epst = sb.tile([128, 1], F32, name=f"{name}_eps")
    nc.vector.memset(epst[:], EPS)  # dep-free, runs early
    sums = red[:, 0, :]
    ssq = red[:, 1, :]
    s2 = sb.tile([128, n_ch], F32, name=f"{name}_s2")
    nc.vector.tensor_mul(s2[:], sums, sums)
    # nvn = sum^2/n - ssq  (= -n*var)
    nvn = sb.tile([128, n_ch], F32, name=f"{name}_nvn")
    nc.vector.scalar_tensor_tensor(
        nvn[:], s2[:], inv_n, ssq, op0=ALU.mult, op1=ALU.subtract
    )
    # std = sqrt(nvn * (-1/n) + eps) = sqrt(var + eps)
    std = sb.tile([128, n_ch], F32, name=f"{name}_std")
    nc.scalar.activation(std[:], nvn[:], ACTF.Sqrt, bias=epst[:, 0:1],
                         scale=-inv_n)
    rstd = sb.tile([128, n_ch], F32, name=f"{name}_rstd")
    nc.vector.reciprocal(rstd[:], std[:])
    a = sb.tile([128, n_ch], F32, name=f"{name}_a")
    nc.vector.tensor_mul(a[:], gamma[:], rstd[:])
    asum = sb.tile([128, n_ch], F32, name=f"{name}_asum")
    nc.vector.tensor_mul(asum[:], a[:], sums)
    # b = beta - a*sum/n
    b = sb.tile([128, n_ch], F32, name=f"{name}_b")
    nc.vector.scalar_tensor_tensor(
        b[:], asum[:], -inv_n, beta[:], op0=ALU.mult, op1=ALU.add
    )
    return a, b


def build_debug(stage):
    """Structural-bisect builds: stage '0' = DMA passthrough only,
    '1' = +conv1/BN1, '2' = +conv2/BN2."""
    nc = bacc.Bacc("TRN2", target_bir_lowering=False, debug=False,
                   num_devices=N_CORES)
    x_d = nc.dram_tensor("x", [NL, CIN, HW], F32, kind="ExternalInput")
    w1_d = nc.dram_tensor("w1t", [CI_CH, 128, P], BF16, kind="ExternalInput")
    w2_d = nc.dram_tensor("w2t", [P_CH, 128, 9, P], BF16, kind="ExternalInput")
    w3_d = nc.dram_tensor("w3t", [P_CH, 128, COUT], BF16, kind="ExternalInput")
    gb1_d = nc.dram_tensor("gb1", [2, P_CH, 128], F32, kind="ExternalInput")
    gb2_d = nc.dram_tensor("gb2", [2, P_CH, 128], F32, kind="ExternalInput")
    gb3_d = nc.dram_tensor("gb3", [2, CO_CH, 128], F32, kind="ExternalInput")
    out_d = nc.dram_tensor("out", [NL, CIN, HW], F32, kind="ExternalOutput")

    with tile.TileContext(nc) as tc, ExitStack() as ctx:
        xpool = ctx.enter_context(tc.tile_pool(name="xpool", bufs=1))
        consts = ctx.enter_context(tc.tile_pool(name="consts", bufs=1))
        statp = ctx.enter_context(tc.tile_pool(name="statp", bufs=1))
        scrp = ctx.enter_context(tc.tile_pool(name="scrp", bufs=2))
        psum = ctx.enter_context(tc.tile_pool(name="psum", bufs=8, space="PSUM"))

        xf = [xpool.tile([128, F], F32, name=f"xf{c}") for c in range(CI_CH)]
        for c in range(CI_CH):
            nc.sync.dma_start(
                xf[c][:], x_d[:, c * 128:(c + 1) * 128, :].rearrange("n p f -> p n f")
            )
        if stage == "0":
            for c in range(CI_CH):
                nc.sync.dma_start(
                    out_d[:, c * 128:(c + 1) * 128, :].rearrange("n p f -> p n f"),
                    xf[c][:],
                )
        elif stage == "1d":
            # no matmuls: bf16 convert then ACT copy back to f32, DMA out
            xb = [xpool.tile([128, F], BF16, name=f"xb{c}") for c in range(CI_CH)]
            zz = [xpool.tile([128, F], F32, name=f"zz{c}") for c in range(CI_CH)]
            for c in range(CI_CH):
                if c % 2 == 0:
                    nc.vector.tensor_copy(xb[c][:], xf[c][:])
                else:
                    nc.scalar.copy(xb[c][:], xf[c][:])
                nc.scalar.copy(zz[c][:], xb[c][:])
                nc.sync.dma_start(
                    out_d[:, c * 128:(c + 1) * 128, :].rearrange("n p f -> p n f"),
                    zz[c][:],
                )
        elif stage == "1c":
            # conv1 with NON-interleaved accumulation groups
            w1sb = [consts.tile([128, P], BF16, name=f"w1sb{c}")
                    for c in range(CI_CH)]
            for c in range(CI_CH):
                nc.sync.dma_start(w1sb[c][:], w1_d[c])
            xb = [xpool.tile([128, F], BF16, name=f"xb{c}") for c in range(CI_CH)]
            for c in range(CI_CH):
                if c % 2 == 0:
                    nc.vector.tensor_copy(xb[c][:], xf[c][:])
                else:
                    nc.scalar.copy(xb[c][:], xf[c][:])
            z1 = [xpool.tile([128, F], F32, name=f"z1_{c}") for c in range(P_CH)]
            for co in range(P_CH):
                for ft in range(FT):
                    pt = psum.tile([128, FTS], F32, name="pt", tag="pt")
                    for ci in range(CI_CH):
                        nc.tensor.matmul(
                            pt[:],
                            w1sb[ci][:, co * 128:(co + 1) * 128],
                            xb[ci][:, ft * FTS:(ft + 1) * FTS],
                            start=(ci == 0), stop=(ci == CI_CH - 1),
                        )
                    nc.scalar.activation(
                        z1[co][:, ft * FTS:(ft + 1) * FTS], pt[:], ACTF.Copy)
            for c in range(P_CH):
                nc.sync.dma_start(
                    out_d[:, c * 128:(c + 1) * 128, :].rearrange("n p f -> p n f"),
                    z1[c][:],
                )
        elif stage in ("1a", "1b", "1e", "1f"):
            # sequential-group conv1 plus one feature at a time:
            # 1a: plain ACT copy out (== 1c)
            # 1b: + ACT accum_out sums
            # 1e: + DVE TTR sumsq
            # 1f: + stats reduce (no collective, no BN)
            w1sb = [consts.tile([128, P], BF16, name=f"w1sb{c}")
                    for c in range(CI_CH)]
            for c in range(CI_CH):
                nc.sync.dma_start(w1sb[c][:], w1_d[c])
            xb = [xpool.tile([128, F], BF16, name=f"xb{c}") for c in range(CI_CH)]
            for c in range(CI_CH):
                if c % 2 == 0:
                    nc.vector.tensor_copy(xb[c][:], xf[c][:])
                else:
                    nc.scalar.copy(xb[c][:], xf[c][:])
            z1 = [xpool.tile([128, F], F32, name=f"z1_{c}") for c in range(P_CH)]
            s1p = statp.tile([128, P_CH, FT], F32, name="s1p")
            q1p = statp.tile([128, P_CH, FT], F32, name="q1p")
            for co in range(P_CH):
                for ft in range(FT):
                    pt = psum.tile([128, FTS], F32, name="pt", tag="pt")
                    for ci in range(CI_CH):
                        nc.tensor.matmul(
                            pt[:],
                            w1sb[ci][:, co * 128:(co + 1) * 128],
                            xb[ci][:, ft * FTS:(ft + 1) * FTS],
                            start=(ci == 0), stop=(ci == CI_CH - 1),
                        )
                    zs = z1[co][:, ft * FTS:(ft + 1) * FTS]
                    if stage == "1a":
                        nc.scalar.activation(zs, pt[:], ACTF.Copy)
                    else:
                        nc.scalar.activation(
                            zs, pt[:], ACTF.Copy,
                            accum_out=s1p[:, co, ft:ft + 1])
                    if stage in ("1e", "1f"):
                        sq = scrp.tile([128, FTS], F32, name="sq", tag="sq")
                        nc.vector.tensor_mul(sq[:], zs, zs)
                        nc.vector.tensor_reduce(
                            q1p[:, co, ft:ft + 1], sq[:], axis=AX.X, op=ALU.add)
            if stage == "1f":
                st1 = statp.tile([128, 2, P_CH], F32, name="st1")
                nc.vector.tensor_reduce(st1[:, 0, :], s1p[:], axis=AX.X,
                                        op=ALU.add)
                nc.vector.tensor_reduce(st1[:, 1, :], q1p[:], axis=AX.X,
                                        op=ALU.add)
                stout = xpool.tile([128, 4], F32, name="stout")
                nc.vector.tensor_copy(stout[:], st1.rearrange("p s c -> p (s c)"))
            for c in range(P_CH):
                nc.sync.dma_start(
                    out_d[:, c * 128:(c + 1) * 128, :].rearrange("n p f -> p n f"),
                    z1[c][:],
                )
        elif stage == "1":
            # conv1 + stats + BN1 apply, result written to out chunks 0..1
            w1sb = [consts.tile([128, P], BF16, name=f"w1sb{c}")
                    for c in range(CI_CH)]
            for c in range(CI_CH):
                nc.sync.dma_start(w1sb[c][:], w1_d[c])
            g1 = consts.tile([128, P_CH], F32, name="g1")
            be1 = consts.tile([128, P_CH], F32, name="be1")
            nc.sync.dma_start(g1[:], gb1_d[0].rearrange("c p -> p c"))
            nc.sync.dma_start(be1[:], gb1_d[1].rearrange("c p -> p c"))
            xb = [xpool.tile([128, F], BF16, name=f"xb{c}") for c in range(CI_CH)]
            for c in range(CI_CH):
                if c % 2 == 0:
                    nc.vector.tensor_copy(xb[c][:], xf[c][:])
                else:
                    nc.scalar.copy(xb[c][:], xf[c][:])
            z1 = [xpool.tile([128, F], F32, name=f"z1_{c}") for c in range(P_CH)]
            s1p = statp.tile([128, P_CH, FT], F32, name="s1p")
            q1p = statp.tile([128, P_CH, FT], F32, name="q1p")
            for co in range(P_CH):
                for ft in range(FT):
                    pt = psum.tile([128, FTS], F32, name="pt", tag="pt")
                    for ci in range(CI_CH):
                        nc.tensor.matmul(
                            pt[:],
                            w1sb[ci][:, co * 128:(co + 1) * 128],
                            xb[ci][:, ft * FTS:(ft + 1) * FTS],
                            start=(ci == 0), stop=(ci == CI_CH - 1),
                        )
                    zs = z1[co][:, ft * FTS:(ft + 1) * FTS]
                    nc.scalar.activation(
                        zs, pt[:], ACTF.Copy, accum_out=s1p[:, co, ft:ft + 1])
                    sq = scrp.tile([128, FTS], F32, name="sq", tag="sq")
                    nc.vector.tensor_mul(sq[:], zs, zs)
                    nc.vector.tensor_reduce(
                        q1p[:, co, ft:ft + 1], sq[:], axis=AX.X, op=ALU.add)
            st1 = statp.tile([128, 2, P_CH], F32, name="st1")
            nc.vector.tensor_reduce(st1[:, 0, :], s1p[:], axis=AX.X, op=ALU.add)
            nc.vector.tensor_reduce(st1[:, 1, :], q1p[:], axis=AX.X, op=ALU.add)
            red1 = _emit_stats_exchange(nc, tc, ctx, "bn1", st1, P_CH)
            a1, b1 = _emit_bn_params(nc, tc, ctx, "bn1", red1, g1, be1, P_CH)
            y1 = [xpool.tile([128, F], F32, name=f"y1_{c}") for c in range(P_CH)]
            for c in range(P_CH):
                nc.scalar.activation(y1[c][:], z1[c][:], ACTF.Relu,
                                     bias=b1[:, c:c + 1], scale=a1[:, c:c + 1])
            for c in range(P_CH):
                nc.sync.dma_start(
                    out_d[:, c * 128:(c + 1) * 128, :].rearrange("n p f -> p n f"),
                    y1[c][:],
                )
        elif stage == "2":
            # conv1 -> BN1 -> padded y1p -> conv2 -> BN2(local dma exchange)
            w1sb = [consts.tile([128, P], BF16, name=f"w1sb{c}")
                    for c in range(CI_CH)]
            for c in range(CI_CH):
                nc.sync.dma_start(w1sb[c][:], w1_d[c])
            w2sb = [consts.tile([128, 9, P], BF16, name=f"w2sb{c}")
                    for c in range(P_CH)]
            for c in range(P_CH):
                nc.sync.dma_start(w2sb[c][:], w2_d[c])
            g1 = consts.tile([128, P_CH], F32, name="g1")
            be1 = consts.tile([128, P_CH], F32, name="be1")
            nc.sync.dma_start(g1[:], gb1_d[0].rearrange("c p -> p c"))
            nc.sync.dma_start(be1[:], gb1_d[1].rearrange("c p -> p c"))
            g2 = consts.tile([128, P_CH], F32, name="g2")
            be2 = consts.tile([128, P_CH], F32, name="be2")
            nc.sync.dma_start(g2[:], gb2_d[0].rearrange("c p -> p c"))
            nc.sync.dma_start(be2[:], gb2_d[1].rearrange("c p -> p c"))
            xb = [xpool.tile([128, F], BF16, name=f"xb{c}") for c in range(CI_CH)]
            for c in range(CI_CH):
                if c % 2 == 0:
                    nc.vector.tensor_copy(xb[c][:], xf[c][:])
                else:
                    nc.scalar.copy(xb[c][:], xf[c][:])
            y1p = [xpool.tile([128, NL, PAD, PAD], BF16, name=f"y1p{c}")
                   for c in range(P_CH)]
            for c in range(P_CH):
                nc.gpsimd.memset(y1p[c][:], 0)
            z1 = [xpool.tile([128, F], F32, name=f"z1_{c}") for c in range(P_CH)]
            z2 = [xpool.tile([128, F], F32, name=f"z2_{c}") for c in range(P_CH)]
            s1p = statp.tile([128, P_CH, FT], F32, name="s1p")
            q1p = statp.tile([128, P_CH, FT], F32, name="q1p")
            for co in range(P_CH):
                for ft in range(FT):
                    pt = psum.tile([128, FTS], F32, name="pt", tag="pt")
                    for ci in range(CI_CH):
                        nc.tensor.matmul(
                            pt[:],
                            w1sb[ci][:, co * 128:(co + 1) * 128],
                            xb[ci][:, ft * FTS:(ft + 1) * FTS],
                            start=(ci == 0), stop=(ci == CI_CH - 1),
                        )
                    zs = z1[co][:, ft * FTS:(ft + 1) * FTS]
                    nc.scalar.activation(
                        zs, pt[:], ACTF.Copy, accum_out=s1p[:, co, ft:ft + 1])
                    sq = scrp.tile([128, FTS], F32, name="sq", tag="sq")
                    nc.vector.tensor_mul(sq[:], zs, zs)
                    nc.vector.tensor_reduce(
                        q1p[:, co, ft:ft + 1], sq[:], axis=AX.X, op=ALU.add)
            st1 = statp.tile([128, 2, P_CH], F32, name="st1")
            nc.vector.tensor_reduce(st1[:, 0, :], s1p[:], axis=AX.X, op=ALU.add)
            nc.vector.tensor_reduce(st1[:, 1, :], q1p[:], axis=AX.X, op=ALU.add)
            red1 = _emit_stats_exchange(nc, tc, ctx, "bn1", st1, P_CH)
            a1, b1 = _emit_bn_params(nc, tc, ctx, "bn1", red1, g1, be1, P_CH)
            for c in range(P_CH):
                nc.scalar.activation(
                    y1p[c][:, :, 1:1 + H, 1:1 + W],
                    z1[c].rearrange("p (n h w) -> p n h w", n=NL, h=H, w=W),
                    ACTF.Relu, bias=b1[:, c:c + 1], scale=a1[:, c:c + 1])
            s2p = statp.tile([128, P_CH, FT], F32, name="s2p")
            q2p = statp.tile([128, P_CH, FT], F32, name="q2p")
            for co in range(P_CH):
                for ft in range(FT):
                    pt = psum.tile([128, FTS], F32, name="pt", tag="pt")
                    for ci in range(P_CH):
                        for tap in range(9):
                            ky, kx = divmod(tap, 3)
                            nc.tensor.matmul(
                                pt[:],
                                w2sb[ci][:, tap, co * 128:(co + 1) * 128],
                                y1p[ci][:, ft * IPT:(ft + 1) * IPT,
                                        ky:ky + H, kx:kx + W],
                                start=(ci == 0 and tap == 0),
                                stop=(ci == P_CH - 1 and tap == 8),
                            )
                    zs = z2[co][:, ft * FTS:(ft + 1) * FTS]
                    nc.scalar.activation(
                        zs, pt[:], ACTF.Copy, accum_out=s2p[:, co, ft:ft + 1])
                    sq = scrp.tile([128, FTS], F32, name="sq", tag="sq")
                    nc.vector.tensor_mul(sq[:], zs, zs)
                    nc.vector.tensor_reduce(
                        q2p[:, co, ft:ft + 1], sq[:], axis=AX.X, op=ALU.add)
            st2 = statp.tile([128, 2, P_CH], F32, name="st2")
            nc.vector.tensor_reduce(st2[:, 0, :], s2p[:], axis=AX.X, op=ALU.add)
            nc.vector.tensor_reduce(st2[:, 1, :], q2p[:], axis=AX.X, op=ALU.add)
            red2 = _emit_stats_exchange(nc, tc, ctx, "bn2", st2, P_CH)
            a2, b2 = _emit_bn_params(nc, tc, ctx, "bn2", red2, g2, be2, P_CH)
            y2 = [xpool.tile([128, F], F32, name=f"y2_{c}") for c in range(P_CH)]
            for c in range(P_CH):
                nc.scalar.activation(y2[c][:], z2[c][:], ACTF.Relu,
                                     bias=b2[:, c:c + 1], scale=a2[:, c:c + 1])
            for c in range(P_CH):
                nc.sync.dma_start(
                    out_d[:, c * 128:(c + 1) * 128, :].rearrange("n p f -> p n f"),
                    y2[c][:],
                )
    nc.compile()
    return nc


def build():
    nc = bacc.Bacc("TRN2", target_bir_lowering=False, debug=False,
                   num_devices=N_CORES)

    # ---- I/O -------------------------------------------------------------
    x_d = nc.dram_tensor("x", [NL, CIN, HW], F32, kind="ExternalInput")
    xb_d = nc.dram_tensor("xb16", [CI_CH, 128, F], BF16, kind="ExternalInput")
    w1_d = nc.dram_tensor("w1t", [CI_CH, 128, P], BF16, kind="ExternalInput")
    w2_d = nc.dram_tensor("w2t", [P_CH, 128, 9, P], BF16, kind="ExternalInput")
    w3_d = nc.dram_tensor("w3t", [P_CH, 128, COUT], BF16, kind="ExternalInput")
    gb1_d = nc.dram_tensor("gb1", [2, P_CH, 128], F32, kind="ExternalInput")
    gb2_d = nc.dram_tensor("gb2", [2, P_CH, 128], F32, kind="ExternalInput")
    gb3_d = nc.dram_tensor("gb3", [2, CO_CH, 128], F32, kind="ExternalInput")
    out_d = nc.dram_tensor("out", [NL, CIN, HW], F32, kind="ExternalOutput")

    with tile.TileContext(nc) as tc, ExitStack() as ctx:
        consts = ctx.enter_context(tc.tile_pool(name="consts", bufs=1))
        xpool = ctx.enter_context(tc.tile_pool(name="xpool", bufs=1))
        actp = ctx.enter_context(tc.tile_pool(name="actp", bufs=1))
        statp = ctx.enter_context(tc.tile_pool(name="statp", bufs=1))
        scrp = ctx.enter_context(tc.tile_pool(name="scrp", bufs=2))
        psum = ctx.enter_context(tc.tile_pool(name="psum", bufs=8, space="PSUM"))

        # ---- bf16 x first: it gates conv1, so its DMAs lead the queue ----
        xb = [xpool.tile([128, F], BF16, name=f"xz{c}", tag=f"xz{c}")
              for c in range(CI_CH)]
        for c in range(CI_CH):
            nc.sync.dma_start(xb[c][:], xb_d[c])

        # ---- weights + BN consts ----------------------------------------
        w1sb = [consts.tile([128, P], BF16, name=f"w1sb{c}") for c in range(CI_CH)]
        for c in range(CI_CH):
            nc.sync.dma_start(w1sb[c][:], w1_d[c])
        w2sb = [consts.tile([128, 9, P], BF16, name=f"w2sb{c}") for c in range(P_CH)]
        for c in range(P_CH):
            nc.sync.dma_start(w2sb[c][:], w2_d[c])
        w3sb = [consts.tile([128, COUT], BF16, name=f"w3sb{c}") for c in range(P_CH)]
        for c in range(P_CH):
            nc.sync.dma_start(w3sb[c][:], w3_d[c])

        g1 = consts.tile([128, P_CH], F32, name="g1")
        be1 = consts.tile([128, P_CH], F32, name="be1")
        g2 = consts.tile([128, P_CH], F32, name="g2")
        be2 = consts.tile([128, P_CH], F32, name="be2")
        g3 = consts.tile([128, CO_CH], F32, name="g3")
        be3 = consts.tile([128, CO_CH], F32, name="be3")
        for t, d in ((g1, gb1_d), (g2, gb2_d), (g3, gb3_d)):
            nc.sync.dma_start(t[:], d[0].rearrange("c p -> p c"))
        for t, d in ((be1, gb1_d), (be2, gb2_d), (be3, gb3_d)):
            nc.sync.dma_start(t[:], d[1].rearrange("c p -> p c"))

        # fp32 x is only needed by the residual tail; emitted after xb and
        # the weights so its bulk DMAs stay off the conv1 critical path.
        xf = [xpool.tile([128, F], F32, name=f"xf{c}") for c in range(CI_CH)]
        for c in range(CI_CH):
            nc.sync.dma_start(
                xf[c][:], x_d[:, c * 128:(c + 1) * 128, :].rearrange("n p f -> p n f")
            )

        # padded bf16 activations for the 3x3 conv: [128, NL, 16, 16]
        y1p = [actp.tile([128, NL, PAD, PAD], BF16, name=f"y1p{c}") for c in range(P_CH)]
        for c in range(P_CH):
            nc.gpsimd.memset(y1p[c][:], 0)

        z1 = [actp.tile([128, F], F32, name=f"z1_{c}") for c in range(P_CH)]
        z2 = [actp.tile([128, F], F32, name=f"z2_{c}") for c in range(P_CH)]
        y2 = [actp.tile([128, F], BF16, name=f"y2_{c}") for c in range(P_CH)]

        # ================= stage A: conv1 (1x1, 1024 -> 256) =============
        # NOTE: accumulation groups are kept sequential per PSUM tile
        # (group-outer, contraction-inner). Interleaving groups across
        # banks (ci-outer emission) hangs on hardware.
        s1p = statp.tile([128, P_CH, FT], F32, name="s1p")
        st1 = statp.tile([128, 2, P_CH], F32, name="st1")
        for co in range(P_CH):
            for ft in range(FT):
                pt = psum.tile([128, FTS], F32, name="pt", tag="pt")
                for ci in range(CI_CH):
                    nc.tensor.matmul(
                        pt[:],
                        w1sb[ci][:, co * 128:(co + 1) * 128],
                        xb[ci][:, ft * FTS:(ft + 1) * FTS],
                        start=(ci == 0),
                        stop=(ci == CI_CH - 1),
                    )
                # PSUM -> SBUF copy with per-channel running sum (DVE), and
                # square with running sum-of-squares (ACT). Each instruction
                # reads PSUM through exactly one non-scalar input (walrus
                # rejects two PSUM reads on one DVE op).
                zs = z1[co][:, ft * FTS:(ft + 1) * FTS]
                nc.scalar.activation(
                    zs, pt[:], ACTF.Copy, accum_out=s1p[:, co, ft:ft + 1],
                )
            sq = scrp.tile([128, F], BF16, name="sq", tag="sq")
            nc.vector.tensor_mul(sq[:], z1[co][:], z1[co][:])
            nc.vector.tensor_reduce(
                st1[:, 1, co:co + 1], sq[:], axis=AX.X, op=ALU.add,
            )
        nc.vector.tensor_reduce(st1[:, 0, :], s1p[:], axis=AX.X, op=ALU.add)

        red1 = _emit_stats_exchange(nc, tc, ctx, "bn1", st1, P_CH)
        a1, b1 = _emit_bn_params(nc, tc, ctx, "bn1", red1, g1, be1, P_CH)

        for c in range(P_CH):
            nc.scalar.activation(
                y1p[c][:, :, 1:1 + H, 1:1 + W],
                z1[c].rearrange("p (n h w) -> p n h w", n=NL, h=H, w=W),
                ACTF.Relu,
                bias=b1[:, c:c + 1],
                scale=a1[:, c:c + 1],
            )

        # ================= stage B: conv2 (3x3, 256 -> 256) ==============
        s2p = statp.tile([128, P_CH, FT], F32, name="s2p")
        st2 = statp.tile([128, 2, P_CH], F32, name="st2")
        for co in range(P_CH):
            for ft in range(FT):
                pt = psum.tile([128, FTS], F32, name="pt", tag="pt")
                for ci in range(P_CH):
                    for tap in range(9):
                        ky, kx = divmod(tap, 3)
                        nc.tensor.matmul(
                            pt[:],
                            w2sb[ci][:, tap, co * 128:(co + 1) * 128],
                            y1p[ci][:, ft * IPT:(ft + 1) * IPT,
                                    ky:ky + H, kx:kx + W],
                            start=(ci == 0 and tap == 0),
                            stop=(ci == P_CH - 1 and tap == 8),
                        )
                zs = z2[co][:, ft * FTS:(ft + 1) * FTS]
                nc.scalar.activation(
                    zs, pt[:], ACTF.Copy, accum_out=s2p[:, co, ft:ft + 1],
                )
            sq = scrp.tile([128, F], BF16, name="sq", tag="sq")
            nc.vector.tensor_mul(sq[:], z2[co][:], z2[co][:])
            nc.vector.tensor_reduce(
                st2[:, 1, co:co + 1], sq[:], axis=AX.X, op=ALU.add,
            )
        nc.vector.tensor_reduce(st2[:, 0, :], s2p[:], axis=AX.X, op=ALU.add)

        red2 = _emit_stats_exchange(nc, tc, ctx, "bn2", st2, P_CH)
        a2, b2 = _emit_bn_params(nc, tc, ctx, "bn2", red2, g2, be2, P_CH)

        for c in range(P_CH):
            nc.scalar.activation(
                y2[c][:], z2[c][:], ACTF.Relu,
                bias=b2[:, c:c + 1], scale=a2[:, c:c + 1],
            )

        # ================= stage C: conv3 (1x1, 256 -> 1024) =============
        s3p = statp.tile([128, CO_CH, FT], F32, name="s3p")
        st3 = statp.tile([128, 2, CO_CH], F32, name="st3")
        z3 = [xpool.tile([128, F], F32, name=f"xz{c}", tag=f"xz{c}")
              for c in range(CO_CH)]
        for co in range(CO_CH):
            for ft in range(FT):
                pt = psum.tile([128, FTS], F32, name="pt", tag="pt")
                for ci in range(P_CH):
                    nc.tensor.matmul(
                        pt[:],
                        w3sb[ci][:, co * 128:(co + 1) * 128],
                        y2[ci][:, ft * FTS:(ft + 1) * FTS],
                        start=(ci == 0),
                        stop=(ci == P_CH - 1),
                    )
                zs = z3[co][:, ft * FTS:(ft + 1) * FTS]
                nc.scalar.activation(
                    zs, pt[:], ACTF.Copy, accum_out=s3p[:, co, ft:ft + 1],
                )
            sq = scrp.tile([128, F], BF16, name="sq", tag="sq")
            nc.vector.tensor_mul(sq[:], z3[co][:], z3[co][:])
            nc.vector.tensor_reduce(
                st3[:, 1, co:co + 1], sq[:], axis=AX.X, op=ALU.add,
            )
        nc.vector.tensor_reduce(st3[:, 0, :], s3p[:], axis=AX.X, op=ALU.add)

        red3 = _emit_stats_exchange(nc, tc, ctx, "bn3", st3, CO_CH)
        a3, b3 = _emit_bn_params(nc, tc, ctx, "bn3", red3, g3, be3, CO_CH)

        # tail: out = relu((a3*z3 + x) + b3) in two fused passes per chunk.
        # Result lands in xf (fp32, dead after the STT reads it) so the
        # output DMA stays fp32 while z3 itself is bf16.
        for co in range(CO_CH):
            nc.vector.scalar_tensor_tensor(
                xf[co][:], z3[co][:], a3[:, co:co + 1], xf[co][:],
                op0=ALU.mult, op1=ALU.add,
            )
            nc.scalar.activation(
                xf[co][:], xf[co][:], ACTF.Relu, bias=b3[:, co:co + 1],
            )
            deng = nc.sync if co % 2 == 0 else nc.scalar
            deng.dma_start(
                out_d[:, co * 128:(co + 1) * 128, :].rearrange("n p f -> p n f"),
                xf[co][:],
            )
    nc.compile()
    return nc


_NC_CACHE = None


def _get_nc():
    global _NC_CACHE
    if _NC_CACHE is None:
        stage = os.environ.get("KERNEL_STAGE", "full")
        _NC_CACHE = build() if stage == "full" else build_debug(stage)
    return _NC_CACHE


def _prep_host(w1, w2, w3, g1, be1, g2, be2, g3, be3, residual_scale):
    bf = ml_dtypes.bfloat16
    # conv weights, pre-transposed to [ci, ...] layouts for lhsT
    w1t = np.ascontiguousarray(
        w1.reshape(P, CIN).T.astype(bf)).reshape(CI_CH, 128, P)
    w2t = np.ascontiguousarray(
        w2.transpose(1, 2, 3, 0).astype(bf)).reshape(P_CH, 128, 9, P)
    w3t = np.ascontiguousarray(
        w3.reshape(COUT, P).T.astype(bf)).reshape(P_CH, 128, COUT)
    s = np.float32(np.log1p(np.exp(np.float64(residual_scale[0]))))
    gb1 = np.stack([g1, be1]).astype(np.float32).reshape(2, P_CH, 128)
    gb2 = np.stack([g2, be2]).astype(np.float32).reshape(2, P_CH, 128)
    gb3 = (np.stack([g3, be3]) * s).astype(np.float32).reshape(2, CO_CH, 128)
    return w1t, w2t, w3t, gb1, gb2, gb3


def prepare_in_maps(inputs):
    x = np.asarray(inputs["x"], dtype=np.float32)
    w1t, w2t, w3t, gb1, gb2, gb3 = _prep_host(
        np.asarray(inputs["w1"], np.float32), np.asarray(inputs["w2"], np.float32),
        np.asarray(inputs["w3"], np.float32), np.asarray(inputs["g1"], np.float32),
        np.asarray(inputs["be1"], np.float32), np.asarray(inputs["g2"], np.float32),
        np.asarray(inputs["be2"], np.float32), np.asarray(inputs["g3"], np.float32),
        np.asarray(inputs["be3"], np.float32),
        np.asarray(inputs["residual_scale"], np.float32),
    )
    in_maps = []
    for c in range(N_CORES):
        shard = np.ascontiguousarray(
            x[c * NL:(c + 1) * NL].reshape(NL, CIN, HW))
        xb16 = np.ascontiguousarray(
            shard.transpose(1, 0, 2).astype(ml_dtypes.bfloat16)
        ).reshape(CI_CH, 128, F)
        in_maps.append({
            "x": shard, "xb16": xb16, "w1t": w1t, "w2t": w2t, "w3t": w3t,
            "gb1": gb1, "gb2": gb2, "gb3": gb3,
        })
    return in_maps


def kernel(**inputs):
    in_maps = prepare_in_maps(inputs)
    nc = _get_nc()
    trace = bool(int(os.environ.get("KERNEL_PROFILE", "0")))
    try:
        res = run_bass_kernel_spmd(nc, in_maps, list(range(N_CORES)), trace=trace)
    except ModuleNotFoundError:
        # axon NTFF profile hook unavailable in this container
        res = run_bass_kernel_spmd(nc, in_maps, list(range(N_CORES)), trace=False)
    if trace:
        kernel.last_exec_time_ns = getattr(res, "exec_time_ns", None)
        kernel.last_profile = res
    out = np.concatenate([res.results[c]["out"] for c in range(N_CORES)], axis=0)
    return out.reshape(N, CIN, H, W)



# revision 17
# speedup vs baseline: 1.3304x; 1.0673x over previous
"""Trainium2 Bass kernel for a ResNet bottleneck block (training-mode BN).

Computes, for x of shape (64, 1024, 14, 14):
    y1 = relu(bn(conv1x1(x, w1)))        # 1024 -> 256
    y2 = relu(bn(conv3x3(y1, w2)))       # 256 -> 256, pad 1
    z3 = bn(conv1x1(y2, w3))             # 256 -> 1024
    out = relu(x + softplus(residual_scale) * z3)

BN is training-mode: per-channel mean/var over (N, H, W) of the full batch.

Sharding: data-parallel over the batch dim, 8 images per core. BN batch
statistics are exact: per-core (sum, sumsq) vectors are AllGather-ed and
reduced on every core.

Pipelining structure (v2): each BN stats exchange is split per channel
chunk so the AllGather latency overlaps matmul work, and every conv is
emitted in two passes over the contraction dim: passA computes a partial
sum over the first half of the input channels (gated only on the first
half's BN apply), copies it to SBUF; passB computes the second half in a
fresh PSUM group and a fused DVE add merges partial + PSUM while also
accumulating per-channel sums. All PSUM accumulation groups stay
contiguous (interleaved groups hang on HW).

Per-tile stats: the PSUM->SBUF copy carries accum_out (per-channel sums);
sum-of-squares comes from a fused tensor_tensor_reduce on the copied tile.

Conv bias is dropped (training BN is invariant to it); softplus(
residual_scale) is folded into gamma3/beta3 on the host. Matmuls run in
bf16 (fp32 PSUM). The residual x is taken from the bf16 copy of x.
"""

import os
import numpy as np
import ml_dtypes
from contextlib import ExitStack

import concourse.bass as bass
import concourse.bacc as bacc
import concourse.mybir as mybir
import concourse.tile as tile
from concourse.bass_utils import run_bass_kernel_spmd

F32 = mybir.dt.float32
BF16 = mybir.dt.bfloat16
AX = mybir.AxisListType
ALU = mybir.AluOpType
ACTF = mybir.ActivationFunctionType

N_CORES = 8
N, CIN, H, W = 64, 1024, 14, 14
P = 256
COUT = 1024
NL = N // N_CORES          # images per core (8)
HW = H * W                 # 196
F = NL * HW                # free positions per core (1568)
FT = 4                     # free-dim tiles
FTS = F // FT              # 392 positions per tile (= 2 images)
IPT = NL // FT             # images per free tile (2)
CI_CH = CIN // 128         # 8
P_CH = P // 128            # 2
CO_CH = COUT // 128        # 8
EPS = 1e-5
COUNT = N * HW             # global positions per channel (12544)
INV_N = 1.0 / COUNT
PAD = 16                   # padded spatial stride (16x16 per image)


def build():
    nc = bacc.Bacc("TRN2", target_bir_lowering=False, debug=False,
                   num_devices=N_CORES)

    # ---- I/O -------------------------------------------------------------
    xb_d = nc.dram_tensor("xb16", [CI_CH, 128, F], BF16, kind="ExternalInput")
    w1_d = nc.dram_tensor("w1t", [CI_CH, 128, P], BF16, kind="ExternalInput")
    w2_d = nc.dram_tensor("w2t", [P_CH, 128, 9, P], BF16, kind="ExternalInput")
    w3_d = nc.dram_tensor("w3t", [P_CH, 128, COUT], BF16, kind="ExternalInput")
    gb1_d = nc.dram_tensor("gb1", [2, P_CH, 128], F32, kind="ExternalInput")
    gb2_d = nc.dram_tensor("gb2", [2, P_CH, 128], F32, kind="ExternalInput")
    gb3_d = nc.dram_tensor("gb3", [2, CO_CH, 128], F32, kind="ExternalInput")
    out_d = nc.dram_tensor("out", [NL, CIN, HW], F32, kind="ExternalOutput")

    with tile.TileContext(nc) as tc, ExitStack() as ctx:
        consts = ctx.enter_context(tc.tile_pool(name="consts", bufs=1))
        xpool = ctx.enter_context(tc.tile_pool(name="xpool", bufs=1))
        actp = ctx.enter_context(tc.tile_pool(name="actp", bufs=1))
        papool = ctx.enter_context(tc.tile_pool(name="papool", bufs=1))
        statp = ctx.enter_context(tc.tile_pool(name="statp", bufs=1))
        scrp = ctx.enter_context(tc.tile_pool(name="scrp", bufs=2))
        outp = ctx.enter_context(tc.tile_pool(name="outp", bufs=4))
        dram = ctx.enter_context(tc.tile_pool(name="ccdram", bufs=1,
                                              space="DRAM"))
        psum = ctx.enter_context(tc.tile_pool(name="psum", bufs=8,
                                              space="PSUM"))

        # ---- input DMAs: x gates conv1, split across both HWDGE rings ----
        xb = [xpool.tile([128, F], BF16, name=f"xb{c}") for c in range(CI_CH)]
        # w1 first on the scalar ring (tiny, gates the first matmul)
        w1sb = consts.tile([128, CI_CH, P], BF16, name="w1sb")
        for c in range(CI_CH):
            nc.scalar.dma_start(w1sb[:, c], w1_d[c])
        for c in range(CI_CH):
            eng = nc.sync if c % 2 == 0 else nc.scalar
            eng.dma_start(xb[c][:], xb_d[c])
        w2sb = consts.tile([128, P_CH, 9, P], BF16, name="w2sb")
        for c in range(P_CH):
            nc.scalar.dma_start(w2sb[:, c], w2_d[c])
        w3sb = consts.tile([128, P_CH, COUT], BF16, name="w3sb")
        for c in range(P_CH):
            nc.scalar.dma_start(w3sb[:, c], w3_d[c])

        g1 = consts.tile([128, P_CH], F32, name="g1")
        be1 = consts.tile([128, P_CH], F32, name="be1")
        g2 = consts.tile([128, P_CH], F32, name="g2")
        be2 = consts.tile([128, P_CH], F32, name="be2")
        g3 = consts.tile([128, CO_CH], F32, name="g3")
        be3 = consts.tile([128, CO_CH], F32, name="be3")
        for t, d in ((g1, gb1_d), (g2, gb2_d), (g3, gb3_d)):
            nc.scalar.dma_start(t[:], d[0].rearrange("c p -> p c"))
        for t, d in ((be1, gb1_d), (be2, gb2_d), (be3, gb3_d)):
            nc.scalar.dma_start(t[:], d[1].rearrange("c p -> p c"))

        epst = consts.tile([128, 1], F32, name="epst")
        nc.vector.memset(epst[:], EPS)

        # padded bf16 activations for the 3x3 conv: [128, NL, 16, 16]
        y1p = [actp.tile([128, NL, PAD, PAD], BF16, name=f"y1p{c}")
               for c in range(P_CH)]
        for c in range(P_CH):
            nc.gpsimd.memset(y1p[c][:], 0)

        # conv partials (passA outputs); reused across convs via tags
        pa = [papool.tile([128, F], BF16, name=f"pa{c}", tag=f"pa{c}")
              for c in range(4)]

        z1 = [actp.tile([128, F], F32, name=f"z1_{c}") for c in range(P_CH)]
        z2 = [actp.tile([128, F], F32, name=f"z2_{c}") for c in range(P_CH)]
        y2 = [actp.tile([128, F], BF16, name=f"y2_{c}") for c in range(P_CH)]
        z3 = [actp.tile([128, F], BF16, name=f"z3_{c}") for c in range(CO_CH)]

        # ---- stats exchange helpers -------------------------------------
        def emit_stats_dma_ag(name, stc, n_ch):
            """DMA [128, 2, n_ch] stats to DRAM and AllGather them."""
            cc_mode = os.environ.get("KERNEL_CC_MODE", "ag")
            cc_in = dram.tile([2, n_ch, 128], F32, name=f"{name}_in")
            cc_out = dram.tile([N_CORES, 2, n_ch, 128], F32,
                               addr_space="Shared" if cc_mode == "ag" else "Local",
                               name=f"{name}_out")
            nc.sync.dma_start(cc_in.rearrange("s c p -> p s c"), stc[:])
            if os.environ.get("KERNEL_CC_MODE", "ag") == "ag":
                nc.gpsimd.collective_compute(
                    "AllGather", ALU.bypass,
                    replica_groups=[list(range(N_CORES))],
                    ins=[cc_in.opt()], outs=[cc_out.opt()],
                )
            else:
                # debug: replicate local stats (wrong numerics, no cc)
                for r in range(N_CORES):
                    nc.sync.dma_start(cc_out[r], cc_in[:])
            return cc_out

        def emit_gather_params(name, cc_out, n_ch, g_ap, be_ap):
            """Pull gathered stats, reduce over cores, compute (a, b)."""
            gath = statp.tile([128, N_CORES, 2 * n_ch], F32, name=f"{name}_g")
            nc.sync.dma_start(gath[:], cc_out.rearrange("r s c p -> p r (s c)"))
            red = statp.tile([128, 2, n_ch], F32, name=f"{name}_r")
            nc.vector.tensor_reduce(
                red.rearrange("p s c -> p (s c)"),
                gath.rearrange("p r x -> p x r"), axis=AX.X, op=ALU.add)
            sums = red[:, 0, :]
            ssq = red[:, 1, :]
            s2 = statp.tile([128, n_ch], F32, name=f"{name}_s2")
            nc.vector.tensor_mul(s2[:], sums, sums)
            nv = statp.tile([128, n_ch], F32, name=f"{name}_nv")
            nc.vector.scalar_tensor_tensor(
                nv[:], s2[:], INV_N, ssq, op0=ALU.mult, op1=ALU.subtract)
            std = statp.tile([128, n_ch], F32, name=f"{name}_std")
            nc.scalar.activation(std[:], nv[:], ACTF.Sqrt, bias=epst[:, 0:1],
                                 scale=-INV_N)
            rstd = statp.tile([128, n_ch], F32, name=f"{name}_rs")
            nc.vector.reciprocal(rstd[:], std[:])
            a = statp.tile([128, n_ch], F32, name=f"{name}_a")
            nc.vector.tensor_mul(a[:], g_ap, rstd[:])
            am = statp.tile([128, n_ch], F32, name=f"{name}_am")
            nc.vector.tensor_mul(am[:], a[:], sums)
            b = statp.tile([128, n_ch], F32, name=f"{name}_b")
            nc.vector.scalar_tensor_tensor(
                b[:], am[:], -INV_N, be_ap, op0=ALU.mult, op1=ALU.add)
            return a, b

        def emit_chunk_stats(name, sp, qp, co):
            """Reduce per-tile (sum, sumsq) slots for chunk co, DMA + AG.

            sp: [128, n_ch, 2, FT] (passA/passB halves); qp: [128, n_ch, FT].
            """
            stc = statp.tile([128, 2, 1], F32, name=f"{name}_stc{co}")
            nc.vector.tensor_reduce(
                stc[:, 0, :],
                sp[:, co:co + 1, :, :].rearrange("p c s f -> p c (s f)"),
                axis=AX.X, op=ALU.add)
            nc.vector.tensor_reduce(stc[:, 1, :], qp[:, co:co + 1, :],
                                    axis=AX.X, op=ALU.add)
            return emit_stats_dma_ag(f"{name}{co}", stc, 1)

        # ================= conv1 (1x1, 1024 -> 256), 2-pass ==============
        s1p = statp.tile([128, P_CH, 2, FT], F32, name="s1p")
        q1p = statp.tile([128, P_CH, FT], F32, name="q1p")
        for co in range(P_CH):
            for ft in range(FT):
                pt = psum.tile([128, FTS], F32, name="pt", tag="pt")
                for ci in range(CI_CH // 2):
                    nc.tensor.matmul(
                        pt[:], w1sb[:, ci, co * 128:(co + 1) * 128],
                        xb[ci][:, ft * FTS:(ft + 1) * FTS],
                        start=(ci == 0), stop=(ci == CI_CH // 2 - 1))
                nc.scalar.activation(pa[co][:, ft * FTS:(ft + 1) * FTS],
                                     pt[:], ACTF.Copy,
                                     accum_out=s1p[:, co, 0, ft:ft + 1])
        cc1 = [None, None]
        for co in range(P_CH):
            for ft in range(FT):
                fsl = slice(ft * FTS, (ft + 1) * FTS)
                pt = psum.tile([128, FTS], F32, name="pt", tag="pt")
                for i, ci in enumerate(range(CI_CH // 2, CI_CH)):
                    nc.tensor.matmul(
                        pt[:], w1sb[:, ci, co * 128:(co + 1) * 128],
                        xb[ci][:, fsl], start=(i == 0),
                        stop=(ci == CI_CH - 1))
                tb = scrp.tile([128, FTS], BF16, name="tb", tag="tb")
                nc.scalar.activation(tb[:], pt[:], ACTF.Copy,
                                     accum_out=s1p[:, co, 1, ft:ft + 1])
                zs = z1[co][:, fsl]
                nc.vector.scalar_tensor_tensor(
                    zs, tb[:], 1.0, pa[co][:, fsl], op0=ALU.mult, op1=ALU.add)
                sq = scrp.tile([128, FTS], BF16, name="sq", tag="sq")
                nc.vector.tensor_mul(sq[:], zs, zs)
                nc.vector.tensor_reduce(q1p[:, co, ft:ft + 1], sq[:],
                                        axis=AX.X, op=ALU.add)
            cc1[co] = emit_chunk_stats("bn1", s1p, q1p, co)

        # BN1 chunk 0 -> y1p[0]; conv2 passA runs on it while AG1b flies
        a1, b1 = [None, None], [None, None]
        a1[0], b1[0] = emit_gather_params("bn1c0", cc1[0], 1,
                                          g1[:, 0:1], be1[:, 0:1])
        nc.scalar.activation(
            y1p[0][:, :, 1:1 + H, 1:1 + W],
            z1[0].rearrange("p (n h w) -> p n h w", n=NL, h=H, w=W),
            ACTF.Relu, bias=b1[0][:, 0:1], scale=a1[0][:, 0:1])

        # ================= conv2 (3x3, 256 -> 256), 2-pass ===============
        s2p = statp.tile([128, P_CH, 2, FT], F32, name="s2p")
        q2p = statp.tile([128, P_CH, FT], F32, name="q2p")
        for co in range(P_CH):
            for ft in range(FT):
                pt = psum.tile([128, FTS], F32, name="pt", tag="pt")
                for tap in range(9):
                    ky, kx = divmod(tap, 3)
                    nc.tensor.matmul(
                        pt[:], w2sb[:, 0, tap, co * 128:(co + 1) * 128],
                        y1p[0][:, ft * IPT:(ft + 1) * IPT, ky:ky + H, kx:kx + W],
                        start=(tap == 0), stop=(tap == 8))
                nc.scalar.activation(pa[2 + co][:, ft * FTS:(ft + 1) * FTS],
                                     pt[:], ACTF.Copy,
                                     accum_out=s2p[:, co, 0, ft:ft + 1])
            if co == 0:
                # BN1 chunk 1 lands mid-passA; emit its consumers here so
                # they don't head-of-line block the passA copies above
                a1[1], b1[1] = emit_gather_params("bn1c1", cc1[1], 1,
                                                  g1[:, 1:2], be1[:, 1:2])
                nc.scalar.activation(
                    y1p[1][:, :, 1:1 + H, 1:1 + W],
                    z1[1].rearrange("p (n h w) -> p n h w", n=NL, h=H, w=W),
                    ACTF.Relu, bias=b1[1][:, 0:1], scale=a1[1][:, 0:1])

        cc2 = [None, None]
        for co in range(P_CH):
            for ft in range(FT):
                fsl = slice(ft * FTS, (ft + 1) * FTS)
                pt = psum.tile([128, FTS], F32, name="pt", tag="pt")
                for tap in range(9):
                    ky, kx = divmod(tap, 3)
                    nc.tensor.matmul(
                        pt[:], w2sb[:, 1, tap, co * 128:(co + 1) * 128],
                        y1p[1][:, ft * IPT:(ft + 1) * IPT, ky:ky + H, kx:kx + W],
                        start=(tap == 0), stop=(tap == 8))
                tb = scrp.tile([128, FTS], BF16, name="tb", tag="tb")
                nc.scalar.activation(tb[:], pt[:], ACTF.Copy,
                                     accum_out=s2p[:, co, 1, ft:ft + 1])
                zs = z2[co][:, fsl]
                nc.vector.scalar_tensor_tensor(
                    zs, tb[:], 1.0, pa[2 + co][:, fsl],
                    op0=ALU.mult, op1=ALU.add)
                sq = scrp.tile([128, FTS], BF16, name="sq", tag="sq")
                nc.vector.tensor_mul(sq[:], zs, zs)
                nc.vector.tensor_reduce(q2p[:, co, ft:ft + 1], sq[:],
                                        axis=AX.X, op=ALU.add)
            cc2[co] = emit_chunk_stats("bn2", s2p, q2p, co)

        # BN2 chunk 0 -> y2[0]; conv3 passA runs on it while AG2b flies
        a2, b2 = [None, None], [None, None]
        a2[0], b2[0] = emit_gather_params("bn2c0", cc2[0], 1,
                                          g2[:, 0:1], be2[:, 0:1])
        nc.scalar.activation(y2[0][:], z2[0][:], ACTF.Relu,
                             bias=b2[0][:, 0:1], scale=a2[0][:, 0:1])

        # ================= conv3 (1x1, 256 -> 1024), single pass =========
        # BN2 chunk 1 must land before conv3's accumulation groups start
        a2[1], b2[1] = emit_gather_params("bn2c1", cc2[1], 1,
                                          g2[:, 1:2], be2[:, 1:2])
        nc.scalar.activation(y2[1][:], z2[1][:], ACTF.Relu,
                             bias=b2[1][:, 0:1], scale=a2[1][:, 0:1])

        s3p = statp.tile([128, CO_CH, FT], F32, name="s3p")
        q3p = statp.tile([128, CO_CH, FT], F32, name="q3p")
        st3 = [statp.tile([128, 2, 4], F32, name=f"st3_{h}") for h in range(2)]
        cc3 = [None, None]
        for co in range(CO_CH):
            for ft in range(FT):
                fsl = slice(ft * FTS, (ft + 1) * FTS)
                pt = psum.tile([128, FTS], F32, name="pt", tag="pt")
                for ci in range(P_CH):
                    nc.tensor.matmul(
                        pt[:], w3sb[:, ci, co * 128:(co + 1) * 128],
                        y2[ci][:, fsl], start=(ci == 0), stop=(ci == P_CH - 1))
                zs = z3[co][:, fsl]
                nc.scalar.activation(zs, pt[:], ACTF.Copy,
                                     accum_out=s3p[:, co, ft:ft + 1])
                sq = scrp.tile([128, FTS], BF16, name="sq", tag="sq")
                nc.vector.tensor_mul(sq[:], zs, zs)
                nc.vector.tensor_reduce(q3p[:, co, ft:ft + 1], sq[:],
                                        axis=AX.X, op=ALU.add)
            if co % 4 == 3:
                h = co // 4
                nc.vector.tensor_reduce(st3[h][:, 0, :],
                                        s3p[:, 4 * h:4 * h + 4, :],
                                        axis=AX.X, op=ALU.add)
                nc.vector.tensor_reduce(st3[h][:, 1, :],
                                        q3p[:, 4 * h:4 * h + 4, :],
                                        axis=AX.X, op=ALU.add)
                cc3[h] = emit_stats_dma_ag(f"bn3h{h}", st3[h], 4)

        # ================= BN3 + residual tail ===========================
        for hh in range(2):
            a3, b3 = emit_gather_params(f"bn3c{hh}", cc3[hh], 4,
                                        g3[:, 4 * hh:4 * hh + 4],
                                        be3[:, 4 * hh:4 * hh + 4])
            for j in range(4):
                co = 4 * hh + j
                t = scrp.tile([128, F], BF16, name="tt", tag="tt")
                nc.vector.scalar_tensor_tensor(
                    t[:], z3[co][:], a3[:, j:j + 1], xb[co][:],
                    op0=ALU.mult, op1=ALU.add)
                ob = outp.tile([128, F], F32, name="ob", tag="ob")
                nc.scalar.activation(ob[:], t[:], ACTF.Relu,
                                     bias=b3[:, j:j + 1])
                deng = nc.sync if co % 2 == 0 else nc.scalar
                deng.dma_start(
                    out_d[:, co * 128:(co + 1) * 128, :].rearrange(
                        "n p f -> p n f"), ob[:])
    nc.compile()
    return nc


_NC_CACHE = None


def _get_nc():
    global _NC_CACHE
    if _NC_CACHE is None:
        _NC_CACHE = build()
    return _NC_CACHE


def _prep_host(w1, w2, w3, g1, be1, g2, be2, g3, be3, residual_scale):
    bf = ml_dtypes.bfloat16
    # conv weights, pre-transposed to [ci, ...] layouts for lhsT
    w1t = np.ascontiguousarray(
        w1.reshape(P, CIN).T.astype(bf)).reshape(CI_CH, 128, P)
    w2t = np.ascontiguousarray(
        w2.transpose(1, 2, 3, 0).astype(bf)).reshape(P_CH, 128, 9, P)
    w3t = np.ascontiguousarray(
        w3.reshape(COUT, P).T.astype(bf)).reshape(P_CH, 128, COUT)
    s = np.float32(np.log1p(np.exp(np.float64(residual_scale[0]))))
    gb1 = np.stack([g1, be1]).astype(np.float32).reshape(2, P_CH, 128)
    gb2 = np.stack([g2, be2]).astype(np.float32).reshape(2, P_CH, 128)
    gb3 = (np.stack([g3, be3]) * s).astype(np.float32).reshape(2, CO_CH, 128)
    return w1t, w2t, w3t, gb1, gb2, gb3


def prepare_in_maps(inputs):
    x = np.asarray(inputs["x"], dtype=np.float32)
    w1t, w2t, w3t, gb1, gb2, gb3 = _prep_host(
        np.asarray(inputs["w1"], np.float32), np.asarray(inputs["w2"], np.float32),
        np.asarray(inputs["w3"], np.float32), np.asarray(inputs["g1"], np.float32),
        np.asarray(inputs["be1"], np.float32), np.asarray(inputs["g2"], np.float32),
        np.asarray(inputs["be2"], np.float32), np.asarray(inputs["g3"], np.float32),
        np.asarray(inputs["be3"], np.float32),
        np.asarray(inputs["residual_scale"], np.float32),
    )
    in_maps = []
    for c in range(N_CORES):
        shard = x[c * NL:(c + 1) * NL].reshape(NL, CIN, HW)
        xb16 = np.ascontiguousarray(
            shard.transpose(1, 0, 2).astype(ml_dtypes.bfloat16)
        ).reshape(CI_CH, 128, F)
        in_maps.append({
            "xb16": xb16, "w1t": w1t, "w2t": w2t, "w3t": w3t,
            "gb1": gb1, "gb2": gb2, "gb3": gb3,
        })
    return in_maps


def kernel(**inputs):
    in_maps = prepare_in_maps(inputs)
    nc = _get_nc()
    trace = bool(int(os.environ.get("KERNEL_PROFILE", "0")))
    try:
        res = run_bass_kernel_spmd(nc, in_maps, list(range(N_CORES)), trace=trace)
    except ModuleNotFoundError:
        # axon NTFF profile hook unavailable in this container
        res = run_bass_kernel_spmd(nc, in_maps, list(range(N_CORES)), trace=False)
    if trace:
        kernel.last_exec_time_ns = getattr(res, "exec_time_ns", None)
        kernel.last_profile = res
    out = np.concatenate([res.results[c]["out"] for c in range(N_CORES)], axis=0)
    return out.reshape(N, CIN, H, W)
